# revision 6
# baseline (speedup 1.0000x reference)
"""Multi-query attention (nn_Attention) Trainium2 Bass kernel, 8-core SPMD.

Reference computation (fp32):
    q = einsum('bnd,hde->bhne', x, Wq) * dh**-0.5
    k, v = split(x @ Wkv)                      # shared across heads (MQA)
    out = softmax(q @ k^T) @ v                 # per head
    out = concat_heads(out) @ Wout

Shapes: x [2,2048,1024], Wq [16,1024,64], Wkv [1024,128], Wout [1024,1024].

Sharding: core = b*4 + g handles batch b and heads [4g, 4g+4). Wout is split
along its input (inner) dim, so each core produces a partial [2048,1024]
output; the host sums the 4 partials per batch.

The matmul datapath is bf16 (weights and activations cast once on write;
all accumulation stays fp32 in PSUM) — measured end-to-end error ~5e-3
against the fp32 reference, well inside the 2e-2 budget.

Per-core pipeline (v2 — tuned against the perfetto trace):
  1. x is transposed on the HOST (free) and lands in SBUF as bf16 xT.
     Group-0 kt-pairs are DMAd from the ACT queue while the weight DMAs
     run on the sync queue, so the first projection matmuls start ~3us
     earlier than a single serialized queue.
  2. Steady-state cycle emits: attn@v of jt-1 (popped from a skew queue
     BEFORE the scores pair, so the pair boundary keeps the exp cadence),
     the row-tiled scores pair for jt, one 1024-elem exp, and at most one
     deferred work item (normalize piece / output-projection group).
  3. The softmax normalize is split into 4 small pieces (2 DVE adds, 2
     bcast+recip+mult tails) deferred into the NEXT pair's cycles; the
     attn@v accumulators live in a 3-slot PSUM ring so the next pair's
     first attn@v never waits on the previous pair's normalize.
  4. qt projections are emitted as two N=256 half-bursts on even cycles.
  5. Tail: the last pair's normalize runs inline; its 8 output-projection
     groups use the (now free) sim PSUM banks, with PSUM->SBUF copies
     alternating DVE/ACT and the final DMAs alternating sync/ACT queues.
"""

import os

import numpy as np
import ml_dtypes

import concourse.mybir as mybir
import concourse.tile as tile
from concourse import bacc
from concourse.bass_utils import run_bass_kernel_spmd
from concourse.dve_ops import RECIP_APPROX_FAST_CONSTS, RECIPROCAL_APPROX_FAST
from concourse.masks import make_identity

DIM = 1024
DIM_HEAD = 64
HEADS = 16
SCALE = DIM_HEAD**-0.5
B = 2
N = 2048
N_CORES = 8
HEADS_PER_CORE = HEADS // 4  # 4 head-groups across cores

P = 128
KT = DIM // P  # 8 contraction tiles
NT = N // P  # 16 row tiles of 128
IT = N // 512  # 4 i-tiles of 512
PAIRS = HEADS_PER_CORE // 2  # 2 head pairs
INNER = HEADS_PER_CORE * DIM_HEAD  # 256 per-core inner dim
CHUNKS = INNER // P  # 2 chunks of the inner dim
WKV_COLS = 192  # [Wk | Wk | Wv]


def _build():
    f32 = mybir.dt.float32
    f32r = mybir.dt.float32r
    bf16 = mybir.dt.bfloat16
    Exp = mybir.ActivationFunctionType.Exp

    nc = bacc.Bacc("TRN2", target_bir_lowering=False, debug=False,
                   enable_asserts=False)

    xt_d = nc.dram_tensor("xt", [DIM, N], bf16, kind="ExternalInput")
    wq_d = nc.dram_tensor("wq", [PAIRS, DIM, P], bf16, kind="ExternalInput")
    wkv_d = nc.dram_tensor("wkv", [DIM, WKV_COLS], bf16, kind="ExternalInput")
    wout_d = nc.dram_tensor("wout", [INNER, DIM], bf16, kind="ExternalInput")
    out_d = nc.dram_tensor("out", [N, DIM], f32, kind="ExternalOutput")

    with tile.TileContext(nc) as tc:
        with (
            tc.tile_pool(name="const", bufs=1) as const,
            tc.tile_pool(name="w", bufs=1) as w,
            tc.tile_pool(name="big", bufs=1) as big,
            tc.tile_pool(name="expp", bufs=6) as expp,
            tc.tile_pool(name="small", bufs=2) as small,
            tc.tile_pool(name="outp", bufs=8) as outp,
            tc.tile_pool(name="ps_small", bufs=1, space="PSUM") as ps_small,
            tc.tile_pool(name="ps_sim", bufs=2, space="PSUM") as ps_sim,
            tc.tile_pool(name="ps_acc", bufs=3, space="PSUM") as ps_acc,
        ):
            identity_f = const.tile([P, P], f32)
            make_identity(nc, identity_f[:])
            identity = const.tile([P, P], f32r)
            nc.vector.tensor_copy(identity[:], identity_f[:])

            xT = big.tile([P, IT, KT, 512], bf16)

            def xt_dma(g):
                gsl = slice(g * 512, (g + 1) * 512)
                nc.sync.dma_start(
                    xT[:, g, :, :],
                    xt_d[:, gsl].rearrange("(ko p) n -> p ko n", p=P),
                )

            def xt0_dma(kt0):
                # group-0 kt-pair DMAs ride the ACT queue, concurrent with
                # the weight DMAs on the sync queue
                nc.scalar.dma_start(
                    xT[:, 0, kt0:kt0 + 2, :],
                    xt_d[kt0 * P:(kt0 + 2) * P, 0:512].rearrange(
                        "(ko p) n -> p ko n", p=P
                    ),
                )

            wkv_sb = w.tile([P, KT, WKV_COLS], bf16)
            wq_sb = w.tile([P, PAIRS, KT, P], bf16)

            xt0_dma(0)
            nc.sync.dma_start(
                wkv_sb[:, 0:4, :],
                wkv_d[0:4 * P, :].rearrange("(ko p) m -> p ko m", p=P),
            )
            xt0_dma(2)
            nc.sync.dma_start(
                wkv_sb[:, 4:8, :],
                wkv_d[4 * P:, :].rearrange("(ko p) m -> p ko m", p=P),
            )
            xt0_dma(4)
            nc.sync.dma_start(
                wq_sb[:, 0, :, :],
                wq_d[0].rearrange("(ko p) m -> p ko m", p=P),
            )
            xt0_dma(6)
            nc.sync.dma_start(
                wq_sb[:, 1, :, :],
                wq_d[1].rearrange("(ko p) m -> p ko m", p=P),
            )
            for g in range(1, IT):
                xt_dma(g)
            wout_sb = w.tile([P, CHUNKS, DIM], bf16)
            nc.sync.dma_start(
                wout_sb[:], wout_d[:].rearrange("(c p) m -> p c m", p=P)
            )

            onescol = const.tile([P, 1], f32)
            nc.gpsimd.memset(onescol[:], 1.0)
            ones65f = const.tile([65, 64], f32)
            nc.gpsimd.memset(ones65f[:], 1.0)
            ones65 = const.tile([65, 64], f32r)
            nc.vector.tensor_copy(ones65[64:65, :], ones65f[64:65, :])

            kT2 = big.tile([P, N], bf16)  # [kT; kT] stacked halves
            vT = big.tile([64, N], f32r)
            v_aug = big.tile([P, NT, 65], bf16)
            nc.vector.tensor_copy(
                v_aug[:, :, 64:65], onescol[:, None, :].to_broadcast((P, NT, 1))
            )
            qT = big.tile([P, PAIRS, N], bf16)
            oTn = big.tile([P, CHUNKS, N], bf16)
            rc = RECIP_APPROX_FAST_CONSTS

            def qt_proj(p, it, half=None, pool=None):
                if half is None:
                    n0, n1 = 0, 512
                else:
                    n0, n1 = half * 256, half * 256 + 256
                isl = slice(it * 512 + n0, it * 512 + n1)
                if pool is not None:
                    simt = pool.tile([P, 2, 512], f32, tag="sim", name="psq_pro")
                    psq = simt[:, 0, n0:n1]
                else:
                    psq = ps_small.tile([P, n1 - n0], f32, tag="pss", name="psq")
                for kt in range(KT):
                    nc.tensor.matmul(
                        psq.opt(),
                        wq_sb[:, p, kt, :],
                        xT[:, it, kt, n0:n1],
                        start=(kt == 0),
                        stop=(kt == KT - 1),
                    )
                nc.vector.tensor_copy(qT[:, p, isl], psq.opt())

            def kv_unit(g):
                isl = slice(g * 512, (g + 1) * 512)
                psk = ps_small.tile([P, 512], f32, tag="pss", name="psk")
                for kt in range(KT):
                    nc.tensor.matmul(
                        psk[:],
                        wkv_sb[:, kt, 0:P],
                        xT[:, g, kt, :],
                        start=(kt == 0),
                        stop=(kt == KT - 1),
                    )
                nc.vector.tensor_copy(kT2[:, isl], psk[:])

            def v_proj(g):
                isl = slice(g * 512, (g + 1) * 512)
                psv = ps_small.tile([64, 512], f32, tag="pss", name="psv")
                for kt in range(KT):
                    nc.tensor.matmul(
                        psv[:],
                        wkv_sb[:, kt, P:P + 64],
                        xT[:, g, kt, :],
                        start=(kt == 0),
                        stop=(kt == KT - 1),
                    )
                nc.vector.tensor_copy(vT[:, isl], psv[:])

            def v_trans(g):
                psvt = ps_small.tile([P, 4, 64], f32r, tag="pss", name="psvt")
                for s in range(4):
                    jt = g * 4 + s
                    nc.tensor.matmul(
                        psvt[:, s, :],
                        vT[:, jt * P:(jt + 1) * P],
                        identity[0:64, 0:64],
                        is_transpose=True,
                        start=(s == 0),
                        stop=(s == 3),
                    )
                nc.vector.tensor_copy(
                    v_aug[:, g * 4:(g + 1) * 4, 0:64], psvt[:]
                )

            pending = []  # deferred normalize pieces + outproj groups
            skewq = []  # deferred attn@v emitters

            def flush_skew(keep=0):
                while len(skewq) > keep:
                    skewq.pop(0)()

            def emit_jt(it, p, po, jt, keep=1, pop=True):
                # attn@v of an earlier jt runs BEFORE this jt's scores so
                # the exp cadence survives pair boundaries and the v_aug
                # weight load prefetches under the previous matmul.
                flush_skew(keep=keep)
                isl = slice(it * 512, (it + 1) * 512)
                jsl = slice(jt * P, (jt + 1) * P)
                pss = ps_sim.tile([P, 2, 512], f32, tag="sim")
                for h in range(2):
                    nc.tensor.matmul(
                        pss[:, h, :],
                        kT2[64 * h:64 * (h + 1), jsl],
                        qT[64 * h:64 * (h + 1), p, isl],
                        tile_position=(64 * h, 0),
                    )
                et = expp.tile([P, 2, 512], bf16, tag="exp")
                nc.scalar.activation(et[:], pss[:], Exp, scale=SCALE)

                def do_oT(po=po, jt=jt, et=et):
                    for h in range(2):
                        nc.tensor.matmul(
                            po[h][:],
                            v_aug[:, jt, :],
                            et[:, h, :],
                            start=(jt % 8 == 0),
                            stop=(jt % 8 == 7),
                        )

                skewq.append(do_oT)
                if pop and pending:
                    pending.pop(0)()

            def alloc_po(it, p, half):
                return [
                    ps_acc.tile(
                        [65, 512], f32, tag="po", name=f"po{h}_{p}_{it}_{half}"
                    )
                    for h in range(2)
                ]

            def flush_half(po, oh):
                for h in range(2):
                    nc.vector.tensor_copy(oh[h][:], po[h][:])

            def alloc_oh(it, p):
                return [
                    small.tile([65, 512], f32r, tag=f"oh{h}", name=f"oh{h}_{p}_{it}")
                    for h in range(2)
                ]

            def norm_tail(it, p, h, ou):
                isl = slice(it * 512, (it + 1) * 512)
                psb = ps_small.tile([64, 512], f32, tag="pss", name="psb")
                nc.tensor.matmul(psb[:], ones65[64:65, :], ou[64:65, :])
                rbc = small.tile([64, 512], f32, tag="rbc")
                nc.vector._custom_dve(
                    RECIPROCAL_APPROX_FAST,
                    out=rbc[:],
                    in0=psb[:],
                    s0=rc["s0"],
                    s1=rc["s1"],
                    imm2=rc["imm2"],
                )
                nc.vector.tensor_tensor(
                    oTn[64 * h:64 * (h + 1), p, isl],
                    ou[0:64, :],
                    rbc[:],
                    mybir.AluOpType.mult,
                )

            def queue_normalize(it, p, oh, po2):
                ous = [None, None]

                def mk_add(h):
                    def f():
                        ou = small.tile(
                            [65, 512], f32r, tag=f"ou{h}", name=f"ou{h}_{p}_{it}"
                        )
                        nc.vector.tensor_tensor(
                            ou[:], oh[h][:], po2[h][:], mybir.AluOpType.add
                        )
                        ous[h] = ou

                    return f

                def mk_tail(h):
                    def f():
                        norm_tail(it, p, h, ous[h])

                    return f

                pending.extend([mk_add(0), mk_add(1), mk_tail(0), mk_tail(1)])

            def outproj_group(itt, dh):
                dsl = slice(dh * 512, (dh + 1) * 512)
                pso = ps_small.tile([P, 512], f32, tag="pss", name="pso")
                for c in range(CHUNKS):
                    nc.tensor.matmul(
                        pso[:],
                        oTn[:, c, itt * P:(itt + 1) * P],
                        wout_sb[:, c, dsl],
                        start=(c == 0),
                        stop=(c == CHUNKS - 1),
                    )
                os_ = outp.tile([P, 512], f32, tag="os")
                nc.vector.tensor_copy(os_[:], pso[:])
                nc.sync.dma_start(out_d[itt * P:(itt + 1) * P, dsl], os_[:])

            def queue_outproj(it):
                for t in range(4):
                    for dh in range(2):
                        pending.append(
                            lambda itt=it * 4 + t, d=dh: outproj_group(itt, d)
                        )

            # ---- Prologue: it=0, pair (0,0), units woven per-jt ----
            kv_unit(0)
            qt_proj(0, 0, pool=ps_sim)
            a0 = alloc_po(0, 0, 0)
            emit_jt(0, 0, a0, 0, keep=3)
            qt_proj(1, 0)
            emit_jt(0, 0, a0, 1, keep=3)
            v_proj(0)
            emit_jt(0, 0, a0, 2, keep=3)
            v_trans(0)
            emit_jt(0, 0, a0, 3, keep=3)
            kv_unit(1)
            emit_jt(0, 0, a0, 4, keep=2)
            v_proj(1)
            emit_jt(0, 0, a0, 5, keep=2)
            v_trans(1)
            emit_jt(0, 0, a0, 6, keep=2)
            kv_unit(2)
            emit_jt(0, 0, a0, 7, keep=2)
            v_proj(2)
            a1 = alloc_po(0, 0, 1)
            emit_jt(0, 0, a1, 8, keep=2)
            v_trans(2)
            emit_jt(0, 0, a1, 9, keep=2)
            kv_unit(3)
            emit_jt(0, 0, a1, 10, keep=2)  # pops attn@v jt=7 -> a0 complete
            ah = alloc_oh(0, 0)
            flush_half(a0, ah)
            v_proj(3)
            emit_jt(0, 0, a1, 11, keep=2)
            v_trans(3)
            emit_jt(0, 0, a1, 12, keep=1)
            qt_proj(0, 1, half=0)
            emit_jt(0, 0, a1, 13, keep=1)
            qt_proj(0, 1, half=1)
            emit_jt(0, 0, a1, 14, keep=1)
            emit_jt(0, 0, a1, 15, keep=1)
            queue_normalize(0, 0, ah, a1)

            # ---- pair (0,1) ----
            b0 = alloc_po(0, 1, 0)
            emit_jt(0, 1, b0, 0, pop=False)
            qt_proj(1, 1, half=0)
            emit_jt(0, 1, b0, 1)
            qt_proj(1, 1, half=1)
            for jt in range(2, 8):
                emit_jt(0, 1, b0, jt)
            b1 = alloc_po(0, 1, 1)
            emit_jt(0, 1, b1, 8)
            emit_jt(0, 1, b1, 9)  # pops attn@v jt=7 -> b0 complete
            bh = alloc_oh(0, 1)
            flush_half(b0, bh)
            for jt in range(10, 16):
                emit_jt(0, 1, b1, jt)
            queue_normalize(0, 1, bh, b1)
            queue_outproj(0)

            # ---- Remaining (it, p) j-loops ----
            seq = [(it, p) for it in range(1, IT) for p in range(PAIRS)]
            for it, p in seq:
                last_pair = (it, p) == (IT - 1, PAIRS - 1)
                po = alloc_po(it, p, 0)
                po2 = None
                oh = None
                for jt in range(16):
                    if jt == 8:
                        po2 = alloc_po(it, p, 1)
                    tgt = po if jt < 8 else po2
                    qt_cycle = it + 1 < IT and jt in (12, 14)
                    emit_jt(it, p, tgt, jt, pop=(jt != 0 and not qt_cycle))
                    if jt == 9:  # attn@v jt=7 has been popped -> po complete
                        oh = alloc_oh(it, p)
                        flush_half(po, oh)
                    if it + 1 < IT:
                        if jt == 12:
                            qt_proj(p, it + 1, half=0)
                        elif jt == 14:
                            qt_proj(p, it + 1, half=1)
                if not last_pair:
                    queue_normalize(it, p, oh, po2)
                    if p == PAIRS - 1:
                        queue_outproj(it)
                else:
                    # ---- Tail: inline normalize + last outproj groups ----
                    flush_skew()
                    ous = []
                    for h in range(2):
                        ou = small.tile(
                            [65, 512], f32r, tag=f"ou{h}", name=f"out{h}_tail"
                        )
                        nc.vector.tensor_tensor(
                            ou[:], oh[h][:], po2[h][:], mybir.AluOpType.add
                        )
                        ous.append(ou)
                    for h in range(2):
                        norm_tail(it, p, h, ous[h])
                    while pending:
                        pending.pop(0)()
                    cur = None
                    for gidx in range(8):
                        t, dh = divmod(gidx, 2)
                        itt = (IT - 1) * 4 + t
                        dsl = slice(dh * 512, (dh + 1) * 512)
                        if gidx % 2 == 0:
                            cur = ps_sim.tile(
                                [P, 2, 512], f32, tag="sim", name=f"tpso{gidx}"
                            )
                        pso = cur[:, gidx % 2, :]
                        for c in range(CHUNKS):
                            nc.tensor.matmul(
                                pso.opt(),
                                oTn[:, c, itt * P:(itt + 1) * P],
                                wout_sb[:, c, dsl],
                                start=(c == 0),
                                stop=(c == CHUNKS - 1),
                            )
                        os_ = outp.tile([P, 512], f32, tag="os")
                        if gidx % 2:
                            nc.scalar.copy(os_[:], pso.opt())
                            nc.scalar.dma_start(
                                out_d[itt * P:(itt + 1) * P, dsl], os_[:]
                            )
                        else:
                            nc.vector.tensor_copy(os_[:], pso.opt())
                            nc.sync.dma_start(
                                out_d[itt * P:(itt + 1) * P, dsl], os_[:]
                            )

    nc.compile()
    return nc


_NC = None


def _get_nc():
    global _NC
    if _NC is None:
        _NC = _build()
    return _NC


def _prep_in_maps(x, Wq, Wkv, Wout):
    in_maps = []
    bf = ml_dtypes.bfloat16
    wk = Wkv[:, 0:DIM_HEAD]
    wv = Wkv[:, DIM_HEAD:]
    wkv_packed = np.ascontiguousarray(
        np.concatenate([wk, wk, wv], axis=1).astype(np.float32).astype(bf)
    )
    for core in range(N_CORES):
        b, g = divmod(core, 4)
        h0 = g * HEADS_PER_CORE
        wq_full = (
            np.transpose(Wq[h0:h0 + HEADS_PER_CORE], (1, 0, 2))
            .reshape(DIM, INNER)
            .astype(np.float32)
            .astype(bf)
        )
        wq = np.ascontiguousarray(
            np.stack([wq_full[:, p * P:(p + 1) * P] for p in range(PAIRS)], 0)
        )
        wout = np.ascontiguousarray(
            Wout[h0 * DIM_HEAD:(h0 + HEADS_PER_CORE) * DIM_HEAD]
            .astype(np.float32)
            .astype(bf)
        )
        in_maps.append(
            {
                "xt": np.ascontiguousarray(
                    x[b].astype(np.float32).astype(bf).T
                ),
                "wq": wq,
                "wkv": wkv_packed,
                "wout": wout,
            }
        )
    return in_maps


def _ensure_hook_shim():
    """bass_utils imports antenv.axon_hooks when tracing is requested via
    env (BASS_TRACE); that module is absent on this image. Provide a no-op
    fallback so an inherited env var cannot break a plain run."""
    try:
        import antenv.axon_hooks  # noqa: F401
    except Exception:
        import sys
        import types

        m = types.ModuleType("antenv.axon_hooks")
        m.get_axon_ntff_profile_hook = lambda: None
        m.set_axon_ntff_profile_hook = lambda h: None
        sys.modules["antenv.axon_hooks"] = m


def run(inputs, trace=False):
    """Run on 8 cores; returns (full_output, BassKernelResults)."""
    _ensure_hook_shim()
    nc = _get_nc()
    in_maps = _prep_in_maps(
        np.asarray(inputs["x"]),
        np.asarray(inputs["Wq"]),
        np.asarray(inputs["Wkv"]),
        np.asarray(inputs["Wout"]),
    )
    res = run_bass_kernel_spmd(
        nc, in_maps, core_ids=list(range(N_CORES)), trace=trace
    )
    out = np.zeros((B, N, DIM), dtype=np.float32)
    for core in range(N_CORES):
        b = core // 4
        out[b] += res.results[core]["out"]
    return out, res


def kernel(**inputs) -> np.ndarray:
    out, _ = run(inputs, trace=bool(os.environ.get("BASS_KERNEL_TRACE")))
    return out


# revision 19
# speedup vs baseline: 1.0596x; 1.0596x over previous
"""Multi-query attention (nn_Attention) Trainium2 Bass kernel, 8-core SPMD.

Reference computation (fp32):
    q = einsum('bnd,hde->bhne', x, Wq) * dh**-0.5
    k, v = split(x @ Wkv)                      # shared across heads (MQA)
    out = softmax(q @ k^T) @ v                 # per head
    out = concat_heads(out) @ Wout

Shapes: x [2,2048,1024], Wq [16,1024,64], Wkv [1024,128], Wout [1024,1024].

Sharding: core = b*4 + g handles batch b and heads [4g, 4g+4). Wout is split
along its input (inner) dim, so each core produces a partial [2048,1024]
output; the host sums the 4 partials per batch.

The matmul datapath is bf16 (weights and activations cast once on write;
all accumulation stays fp32 in PSUM) — measured end-to-end error ~5e-3
against the fp32 reference, well inside the 2e-2 budget.

Per-core pipeline (v2 — tuned against the perfetto trace):
  1. x is transposed on the HOST (free) and lands in SBUF as bf16 xT.
     Group-0 kt-pairs are DMAd from the ACT queue while the weight DMAs
     run on the sync queue, so the first projection matmuls start ~3us
     earlier than a single serialized queue.
  2. Steady-state cycle emits: attn@v of jt-1 (popped from a skew queue
     BEFORE the scores pair, so the pair boundary keeps the exp cadence),
     the row-tiled scores pair for jt, one 1024-elem exp, and at most one
     deferred work item (normalize piece / output-projection group).
  3. The softmax normalize is split into 4 small pieces (2 DVE adds, 2
     bcast+recip+mult tails) deferred into the NEXT pair's cycles; the
     attn@v accumulators live in a 3-slot PSUM ring so the next pair's
     first attn@v never waits on the previous pair's normalize.
  4. qt projections are emitted as two N=256 half-bursts on even cycles.
  5. Tail: the last pair's normalize runs inline; its 8 output-projection
     groups use the (now free) sim PSUM banks, with PSUM->SBUF copies
     alternating DVE/ACT and the final DMAs alternating sync/ACT queues.
"""

import os

import numpy as np
import ml_dtypes

import concourse.mybir as mybir
import concourse.tile as tile
from concourse import bacc
from concourse.bass_utils import run_bass_kernel_spmd
from concourse.dve_ops import RECIP_APPROX_FAST_CONSTS, RECIPROCAL_APPROX_FAST
from concourse.masks import make_identity

DIM = 1024
DIM_HEAD = 64
HEADS = 16
SCALE = DIM_HEAD**-0.5
B = 2
N = 2048
N_CORES = 8
HEADS_PER_CORE = HEADS // 4  # 4 head-groups across cores

P = 128
KT = DIM // P  # 8 contraction tiles
NT = N // P  # 16 row tiles of 128
IT = N // 512  # 4 i-tiles of 512
PAIRS = HEADS_PER_CORE // 2  # 2 head pairs
INNER = HEADS_PER_CORE * DIM_HEAD  # 256 per-core inner dim
CHUNKS = INNER // P  # 2 chunks of the inner dim
WKV_COLS = 256  # [Wk | Wk | Wv | Wk]: cols 0:128 for g=0, 128:256 merged


def _build():
    f32 = mybir.dt.float32
    f32r = mybir.dt.float32r
    bf16 = mybir.dt.bfloat16
    Exp = mybir.ActivationFunctionType.Exp

    nc = bacc.Bacc("TRN2", target_bir_lowering=False, debug=False,
                   enable_asserts=False)

    xt_d = nc.dram_tensor("xt", [DIM, N], bf16, kind="ExternalInput")
    wq_d = nc.dram_tensor("wq", [PAIRS, DIM, P], bf16, kind="ExternalInput")
    wkv_d = nc.dram_tensor("wkv", [DIM, WKV_COLS], bf16, kind="ExternalInput")
    wout_d = nc.dram_tensor("wout", [INNER, DIM], bf16, kind="ExternalInput")
    out_d = nc.dram_tensor("out", [N, DIM], f32, kind="ExternalOutput")

    with tile.TileContext(nc) as tc:
        with (
            tc.tile_pool(name="const", bufs=1) as const,
            tc.tile_pool(name="w", bufs=1) as w,
            tc.tile_pool(name="big", bufs=1) as big,
            tc.tile_pool(name="expp", bufs=6) as expp,
            tc.tile_pool(name="small", bufs=2) as small,
            tc.tile_pool(name="outp", bufs=8) as outp,
            tc.tile_pool(name="ps_small", bufs=1, space="PSUM") as ps_small,
            tc.tile_pool(name="ps_sim", bufs=2, space="PSUM") as ps_sim,
            tc.tile_pool(name="ps_acc", bufs=3, space="PSUM") as ps_acc,
        ):
            identity_f = const.tile([P, P], f32)
            make_identity(nc, identity_f[:])
            identity = const.tile([P, P], f32r)
            nc.vector.tensor_copy(identity[:], identity_f[:])

            xT = big.tile([P, IT, KT, 512], bf16)

            def xt_dma(g):
                gsl = slice(g * 512, (g + 1) * 512)
                nc.sync.dma_start(
                    xT[:, g, :, :],
                    xt_d[:, gsl].rearrange("(ko p) n -> p ko n", p=P),
                )

            def xt0_dma(kt0):
                # group-0 kt-pair DMAs ride the ACT queue, concurrent with
                # the weight DMAs on the sync queue
                nc.scalar.dma_start(
                    xT[:, 0, kt0:kt0 + 2, :],
                    xt_d[kt0 * P:(kt0 + 2) * P, 0:512].rearrange(
                        "(ko p) n -> p ko n", p=P
                    ),
                )

            wkv_sb = w.tile([P, KT, WKV_COLS], bf16)
            wq_sb = w.tile([P, PAIRS, KT, P], bf16)

            xt0_dma(0)
            nc.sync.dma_start(
                wkv_sb[:, 0:4, :],
                wkv_d[0:4 * P, :].rearrange("(ko p) m -> p ko m", p=P),
            )
            xt0_dma(2)
            nc.sync.dma_start(
                wkv_sb[:, 4:8, :],
                wkv_d[4 * P:, :].rearrange("(ko p) m -> p ko m", p=P),
            )
            xt0_dma(4)
            nc.sync.dma_start(
                wq_sb[:, 0, :, :],
                wq_d[0].rearrange("(ko p) m -> p ko m", p=P),
            )
            xt0_dma(6)
            nc.sync.dma_start(
                wq_sb[:, 1, :, :],
                wq_d[1].rearrange("(ko p) m -> p ko m", p=P),
            )
            for g in range(1, IT):
                xt_dma(g)
            wout_sb = w.tile([P, CHUNKS, DIM], bf16)
            nc.sync.dma_start(
                wout_sb[:], wout_d[:].rearrange("(c p) m -> p c m", p=P)
            )

            onescol = const.tile([P, 1], f32)
            nc.gpsimd.memset(onescol[:], 1.0)
            ones65f = const.tile([65, 64], f32)
            nc.gpsimd.memset(ones65f[:], 1.0)
            ones65 = const.tile([65, 64], f32r)
            nc.vector.tensor_copy(ones65[64:65, :], ones65f[64:65, :])

            kT2 = big.tile([P, N], bf16)  # [kT; kT] stacked halves
            vT = big.tile([64, N], f32r)
            v_aug = big.tile([P, NT, 65], bf16)
            nc.vector.tensor_copy(
                v_aug[:, :, 64:65], onescol[:, None, :].to_broadcast((P, NT, 1))
            )
            qT = big.tile([P, PAIRS, N], bf16)
            oTn = big.tile([P, CHUNKS, N], bf16)
            rc = RECIP_APPROX_FAST_CONSTS

            def qt_proj(p, it, half=None, pool=None):
                if half is None:
                    n0, n1 = 0, 512
                else:
                    n0, n1 = half * 256, half * 256 + 256
                isl = slice(it * 512 + n0, it * 512 + n1)
                if pool is not None:
                    simt = pool.tile([P, 2, 512], f32, tag="sim", name="psq_pro")
                    psq = simt[:, 0, n0:n1]
                else:
                    psq = ps_small.tile([P, n1 - n0], f32, tag="pss", name="psq")
                for kt in range(KT):
                    nc.tensor.matmul(
                        psq.opt(),
                        wq_sb[:, p, kt, :],
                        xT[:, it, kt, n0:n1],
                        start=(kt == 0),
                        stop=(kt == KT - 1),
                    )
                nc.vector.tensor_copy(qT[:, p, isl], psq.opt())

            def kv_unit(g):
                # g=0 only: [Wk|Wk] stationary gives both kT2 halves with one
                # partition-aligned copy — keeps the first-exp path short.
                isl = slice(g * 512, (g + 1) * 512)
                psk = ps_small.tile([P, 512], f32, tag="pss", name="psk")
                for kt in range(KT):
                    nc.tensor.matmul(
                        psk[:],
                        wkv_sb[:, kt, 0:P],
                        xT[:, g, kt, :],
                        start=(kt == 0),
                        stop=(kt == KT - 1),
                    )
                nc.vector.tensor_copy(kT2[:, isl], psk[:])

            def v_proj(g):
                # g=0 only: separate Wv chain writing vT rows 0:64
                isl = slice(g * 512, (g + 1) * 512)
                psv = ps_small.tile([64, 512], f32, tag="pss", name="psv")
                for kt in range(KT):
                    nc.tensor.matmul(
                        psv[:],
                        wkv_sb[:, kt, 128:192],
                        xT[:, g, kt, :],
                        start=(kt == 0),
                        stop=(kt == KT - 1),
                    )
                nc.vector.tensor_copy(vT[0:64, isl], psv[:])

            def kvm_unit(g):
                # merged [Wv|Wk] chain (wkv cols 128:256): rows 0:64 = vT,
                # rows 64:128 = kT. One 8-MM chain instead of two. The lower
                # kT2 half is duplicated by an SBUF->SBUF DMA on the sync
                # queue (lands well before this group's scores need it).
                isl = slice(g * 512, (g + 1) * 512)
                psk = ps_small.tile([P, 512], f32, tag="pss", name="pskm")
                for kt in range(KT):
                    nc.tensor.matmul(
                        psk[:],
                        wkv_sb[:, kt, 128:256],
                        xT[:, g, kt, :],
                        start=(kt == 0),
                        stop=(kt == KT - 1),
                    )
                nc.vector.tensor_copy(vT[0:64, isl], psk[0:64, :])
                nc.vector.tensor_copy(kT2[64:128, isl], psk[64:128, :])
                nc.sync.dma_start(kT2[0:64, isl], kT2[64:128, isl])

            def v_trans(g):
                psvt = ps_small.tile([P, 4, 64], f32r, tag="pss", name="psvt")
                for s in range(4):
                    jt = g * 4 + s
                    nc.tensor.matmul(
                        psvt[:, s, :],
                        vT[0:64, jt * P:(jt + 1) * P],
                        identity[0:64, 0:64],
                        is_transpose=True,
                        start=(s == 0),
                        stop=(s == 3),
                    )
                nc.vector.tensor_copy(
                    v_aug[:, g * 4:(g + 1) * 4, 0:64], psvt[:]
                )

            pending = []  # deferred normalize pieces + outproj groups
            skewq = []  # deferred attn@v emitters

            def flush_skew(keep=0):
                while len(skewq) > keep:
                    skewq.pop(0)()

            def emit_jt(it, p, po, jt, keep=2, pop=True):
                # attn@v of an earlier jt runs BEFORE this jt's scores so
                # the exp cadence survives pair boundaries and the v_aug
                # weight load prefetches under the previous matmul.
                flush_skew(keep=keep)
                isl = slice(it * 512, (it + 1) * 512)
                jsl = slice(jt * P, (jt + 1) * P)
                pss = ps_sim.tile([P, 2, 512], f32, tag="sim")
                for h in range(2):
                    nc.tensor.matmul(
                        pss[:, h, :],
                        kT2[64 * h:64 * (h + 1), jsl],
                        qT[64 * h:64 * (h + 1), p, isl],
                        tile_position=(64 * h, 0),
                    )
                et = expp.tile([P, 2, 512], bf16, tag="exp")
                nc.scalar.activation(et[:], pss[:], Exp, scale=SCALE)

                def do_oT(po=po, jt=jt, et=et):
                    for h in range(2):
                        nc.tensor.matmul(
                            po[h][:],
                            v_aug[:, jt, :],
                            et[:, h, :],
                            start=(jt % 8 == 0),
                            stop=(jt % 8 == 7),
                        )

                skewq.append(do_oT)
                if pop and pending:
                    pending.pop(0)()

            def alloc_po(it, p, half):
                return [
                    ps_acc.tile(
                        [65, 512], f32, tag="po", name=f"po{h}_{p}_{it}_{half}"
                    )
                    for h in range(2)
                ]

            def flush_half(po, oh):
                for h in range(2):
                    nc.vector.tensor_copy(oh[h][:], po[h][:])

            def alloc_oh(it, p):
                return [
                    small.tile([65, 512], f32r, tag=f"oh{h}", name=f"oh{h}_{p}_{it}")
                    for h in range(2)
                ]

            def norm_tail(it, p, h, ou):
                isl = slice(it * 512, (it + 1) * 512)
                psb = ps_small.tile([64, 512], f32, tag="pss", name="psb")
                nc.tensor.matmul(psb[:], ones65[64:65, :], ou[64:65, :])
                rbc = small.tile([64, 512], f32, tag="rbc")
                nc.vector._custom_dve(
                    RECIPROCAL_APPROX_FAST,
                    out=rbc[:],
                    in0=psb[:],
                    s0=rc["s0"],
                    s1=rc["s1"],
                    imm2=rc["imm2"],
                )
                nc.vector.tensor_tensor(
                    oTn[64 * h:64 * (h + 1), p, isl],
                    ou[0:64, :],
                    rbc[:],
                    mybir.AluOpType.mult,
                )

            def queue_normalize(it, p, oh, po2):
                ous = [None, None]

                def mk_add(h):
                    def f():
                        ou = small.tile(
                            [65, 512], f32r, tag=f"ou{h}", name=f"ou{h}_{p}_{it}"
                        )
                        nc.vector.tensor_tensor(
                            ou[:], oh[h][:], po2[h][:], mybir.AluOpType.add
                        )
                        ous[h] = ou

                    return f

                def mk_tail(h):
                    def f():
                        norm_tail(it, p, h, ous[h])

                    return f

                pending.extend([mk_add(0), mk_add(1), mk_tail(0), mk_tail(1)])

            def outproj_group(itt, dh):
                dsl = slice(dh * 512, (dh + 1) * 512)
                pso = ps_small.tile([P, 512], f32, tag="pss", name="pso")
                for c in range(CHUNKS):
                    nc.tensor.matmul(
                        pso[:],
                        oTn[:, c, itt * P:(itt + 1) * P],
                        wout_sb[:, c, dsl],
                        start=(c == 0),
                        stop=(c == CHUNKS - 1),
                    )
                os_ = outp.tile([P, 512], f32, tag="os")
                nc.vector.tensor_copy(os_[:], pso[:])
                nc.sync.dma_start(out_d[itt * P:(itt + 1) * P, dsl], os_[:])

            def queue_outproj(it):
                for t in range(4):
                    for dh in range(2):
                        pending.append(
                            lambda itt=it * 4 + t, d=dh: outproj_group(itt, d)
                        )

            # ---- Prologue: it=0, pair (0,0), units woven per-jt ----
            kv_unit(0)
            qt_proj(0, 0, pool=ps_sim)
            a0 = alloc_po(0, 0, 0)
            emit_jt(0, 0, a0, 0, keep=3)
            qt_proj(1, 0)
            emit_jt(0, 0, a0, 1, keep=3)
            kvm_unit(1)
            emit_jt(0, 0, a0, 2, keep=3)
            v_proj(0)
            emit_jt(0, 0, a0, 3, keep=3)
            v_trans(0)
            emit_jt(0, 0, a0, 4, keep=2)
            kvm_unit(2)
            emit_jt(0, 0, a0, 5, keep=2)
            v_trans(1)
            emit_jt(0, 0, a0, 6, keep=2)
            emit_jt(0, 0, a0, 7, keep=2)
            kvm_unit(3)
            a1 = alloc_po(0, 0, 1)
            emit_jt(0, 0, a1, 8, keep=2)
            v_trans(2)
            emit_jt(0, 0, a1, 9, keep=2)
            emit_jt(0, 0, a1, 10, keep=2)  # pops attn@v jt=7 -> a0 complete
            ah = alloc_oh(0, 0)
            flush_half(a0, ah)
            v_trans(3)
            emit_jt(0, 0, a1, 11, keep=2)
            emit_jt(0, 0, a1, 12, keep=2)
            qt_proj(0, 1, half=0)
            emit_jt(0, 0, a1, 13, keep=2)
            emit_jt(0, 0, a1, 14, keep=2)
            qt_proj(0, 1, half=1)
            emit_jt(0, 0, a1, 15, keep=2)
            queue_normalize(0, 0, ah, a1)

            # ---- pair (0,1) ----
            b0 = alloc_po(0, 1, 0)
            emit_jt(0, 1, b0, 0, pop=False)
            qt_proj(1, 1, half=0)
            emit_jt(0, 1, b0, 1, pop=False)
            qt_proj(1, 1, half=1)
            for jt in range(2, 8):
                emit_jt(0, 1, b0, jt)
            b1 = alloc_po(0, 1, 1)
            emit_jt(0, 1, b1, 8)
            emit_jt(0, 1, b1, 9)
            emit_jt(0, 1, b1, 10)  # pops attn@v jt=7 -> b0 complete
            bh = alloc_oh(0, 1)
            flush_half(b0, bh)
            for jt in range(11, 16):
                emit_jt(0, 1, b1, jt)
            queue_normalize(0, 1, bh, b1)
            queue_outproj(0)

            # ---- Remaining (it, p) j-loops ----
            seq = [(it, p) for it in range(1, IT) for p in range(PAIRS)]
            for it, p in seq:
                last_pair = (it, p) == (IT - 1, PAIRS - 1)
                po = alloc_po(it, p, 0)
                po2 = None
                oh = None
                for jt in range(16):
                    if jt == 8:
                        po2 = alloc_po(it, p, 1)
                    tgt = po if jt < 8 else po2
                    qt_cycle = it + 1 < IT and jt in (9, 15)
                    emit_jt(it, p, tgt, jt, pop=(jt >= 2 and not qt_cycle))
                    if jt == 10:  # attn@v jt=7 has been popped -> po complete
                        oh = alloc_oh(it, p)
                        flush_half(po, oh)
                    if it + 1 < IT:
                        if jt == 9:
                            qt_proj(p, it + 1, half=0)
                        elif jt == 15:
                            qt_proj(p, it + 1, half=1)
                if not last_pair:
                    queue_normalize(it, p, oh, po2)
                    if p == PAIRS - 1:
                        queue_outproj(it)
                else:
                    # ---- Tail: inline normalize + last outproj groups.
                    # The chunk-0 matmuls only need pair (3,0)'s (already
                    # normalized) oTn, so they run DURING the normalize chain
                    # — keeping the PE busy (no HAM re-throttle) and off the
                    # critical path. chunk-1 accumulates after the MULTs.
                    flush_skew()
                    while pending:
                        pending.pop(0)()
                    ous = []
                    for h in range(2):
                        ou = small.tile(
                            [65, 512], f32r, tag=f"ou{h}", name=f"out{h}_tail"
                        )
                        nc.vector.tensor_tensor(
                            ou[:], oh[h][:], po2[h][:], mybir.AluOpType.add
                        )
                        ous.append(ou)
                    tail_psos = []
                    for gidx in range(8):
                        if gidx < 4:
                            if gidx % 2 == 0:
                                cur = ps_sim.tile(
                                    [P, 2, 512], f32, tag="sim",
                                    name=f"tpso{gidx}"
                                )
                            tail_psos.append(cur[:, gidx % 2, :])
                        elif gidx < 7:
                            tail_psos.append(ps_acc.tile(
                                [P, 512], f32, tag="po", name=f"tpso{gidx}"
                            ))
                        else:
                            tail_psos.append(ps_small.tile(
                                [P, 512], f32, tag="pss", name=f"tpso{gidx}"
                            ))

                    def tail_c(gidx, c):
                        t, dh = divmod(gidx, 2)
                        itt = (IT - 1) * 4 + t
                        dsl = slice(dh * 512, (dh + 1) * 512)
                        nc.tensor.matmul(
                            tail_psos[gidx].opt(),
                            oTn[:, c, itt * P:(itt + 1) * P],
                            wout_sb[:, c, dsl],
                            start=(c == 0),
                            stop=(c == CHUNKS - 1),
                        )

                    tail_c(0, 0)
                    tail_c(1, 0)
                    norm_tail(it, p, 0, ous[0])
                    tail_c(2, 0)
                    tail_c(3, 0)
                    tail_c(4, 0)
                    norm_tail(it, p, 1, ous[1])
                    tail_c(5, 0)
                    tail_c(6, 0)
                    tail_c(7, 0)
                    for gidx in range(8):
                        t, dh = divmod(gidx, 2)
                        itt = (IT - 1) * 4 + t
                        dsl = slice(dh * 512, (dh + 1) * 512)
                        tail_c(gidx, 1)
                        os_ = outp.tile([P, 512], f32, tag="os")
                        if gidx % 2:
                            nc.scalar.copy(os_[:], tail_psos[gidx].opt())
                            nc.scalar.dma_start(
                                out_d[itt * P:(itt + 1) * P, dsl], os_[:]
                            )
                        else:
                            nc.vector.tensor_copy(os_[:], tail_psos[gidx].opt())
                            nc.sync.dma_start(
                                out_d[itt * P:(itt + 1) * P, dsl], os_[:]
                            )

    nc.compile()
    return nc


_NC = None


def _get_nc():
    global _NC
    if _NC is None:
        _NC = _build()
    return _NC


def _prep_in_maps(x, Wq, Wkv, Wout):
    in_maps = []
    bf = ml_dtypes.bfloat16
    wk = Wkv[:, 0:DIM_HEAD]
    wv = Wkv[:, DIM_HEAD:]
    wkv_packed = np.ascontiguousarray(
        np.concatenate([wk, wk, wv, wk], axis=1).astype(np.float32).astype(bf)
    )
    for core in range(N_CORES):
        b, g = divmod(core, 4)
        h0 = g * HEADS_PER_CORE
        wq_full = (
            np.transpose(Wq[h0:h0 + HEADS_PER_CORE], (1, 0, 2))
            .reshape(DIM, INNER)
            .astype(np.float32)
            .astype(bf)
        )
        wq = np.ascontiguousarray(
            np.stack([wq_full[:, p * P:(p + 1) * P] for p in range(PAIRS)], 0)
        )
        wout = np.ascontiguousarray(
            Wout[h0 * DIM_HEAD:(h0 + HEADS_PER_CORE) * DIM_HEAD]
            .astype(np.float32)
            .astype(bf)
        )
        in_maps.append(
            {
                "xt": np.ascontiguousarray(
                    x[b].astype(np.float32).astype(bf).T
                ),
                "wq": wq,
                "wkv": wkv_packed,
                "wout": wout,
            }
        )
    return in_maps


def _ensure_hook_shim():
    """bass_utils imports antenv.axon_hooks when tracing is requested via
    env (BASS_TRACE); that module is absent on this image. Provide a no-op
    fallback so an inherited env var cannot break a plain run."""
    try:
        import antenv.axon_hooks  # noqa: F401
    except Exception:
        import sys
        import types

        m = types.ModuleType("antenv.axon_hooks")
        m.get_axon_ntff_profile_hook = lambda: None
        m.set_axon_ntff_profile_hook = lambda h: None
        sys.modules["antenv.axon_hooks"] = m


def run(inputs, trace=False):
    """Run on 8 cores; returns (full_output, BassKernelResults)."""
    _ensure_hook_shim()
    nc = _get_nc()
    in_maps = _prep_in_maps(
        np.asarray(inputs["x"]),
        np.asarray(inputs["Wq"]),
        np.asarray(inputs["Wkv"]),
        np.asarray(inputs["Wout"]),
    )
    res = run_bass_kernel_spmd(
        nc, in_maps, core_ids=list(range(N_CORES)), trace=trace
    )
    out = np.zeros((B, N, DIM), dtype=np.float32)
    for core in range(N_CORES):
        b = core // 4
        out[b] += res.results[core]["out"]
    return out, res


def kernel(**inputs) -> np.ndarray:
    out, _ = run(inputs, trace=bool(os.environ.get("BASS_KERNEL_TRACE")))
    return out
